# revision 1
# baseline (speedup 1.0000x reference)
"""GCN (4x SAGEConv mean-agg + PReLU + BatchNorm, graph mean-pool) on 8 TRN2 NeuronCores.

Contract: kernel(**inputs) takes FULL inputs (as produced by setup_inputs) and
returns the FULL [G, 4H] output. Self-contained: all shapes/sharding hardcoded.

Sharding: nodes (and their in-edges, i.e. edges bucketed by dst) are
partitioned contiguously across 8 cores. Weights replicated. h is replicated
in HBM per layer via AllGather. BatchNorm stats and the final pooled output
use small AllReduces.

Aggregation: edges sorted by (dst-range, dst) and packed into 128-edge tiles
of <=32 whole dst runs. Per tile one indirect-DMA gather pulls h[src] rows
(128 x 512B descriptors); a (1/deg-weighted) indicator matmul reduces the tile
to its dst slots in PSUM (3 tiles per PSUM tile at base partitions 0/32/64);
all 1024 slots of a 24-tile call are flushed with ONE direct HWDGE DMA into a
slot-space DRAM buffer, and the main pass reads each 128-node block back with
a single 128-row indirect gather through the host-built inverse slot map
(deg-0 nodes point at a zeroed trailing block). Layer 1 needs no gather or
DRAM roundtrip at all: agg0 is a count-matrix matmul against the 257-row
embedding table, and the self term gathers from the tiny table directly.

The device-time floor on this hardware is the Pool engine's SWDGE fixed cost
(~1us per indirect DMA instruction, max 128 descriptors each); the batched
GPSIMD dma_gather/dma_scatter_add ucode that would lift it is not present in
this (bedrock) image.
"""

import numpy as np

import concourse.bass as bass
import concourse.tile as tile
from concourse import bacc, mybir
from concourse.masks import make_identity

FP = mybir.dt.float32
I16 = mybir.dt.int16
I32 = mybir.dt.int32

N_CORES = 8
P = 128          # partitions
J = 32           # dst slots (runs) per edge-tile
TPP = 3          # edge-tiles per PSUM tile (matmul out base partition 0/32/64)
PPC = 8          # PSUM tiles per call
TPC = TPP * PPC  # 24 edge-tiles per gather/scatter call
NIDX = TPC * P   # 3072 gather indices per call
NSLOT = PPC * P  # 1024 scatter slots per call
L = 4
EPS = 1e-5

# SEG = dst rows per agg range (per core, multiple of 128). Each range gets
# its own agg tensor so Tile can overlap main-pass blocks of completed ranges
# with the remaining ranges' gathers/scatters.
CFG_FULL = dict(N=100_000, G=128, H=128, NV=257, SEG=3_200)


def _mkcfg(N, G, H, NV, SEG):
    assert N % N_CORES == 0
    npc = N // N_CORES
    nblk = (npc + P - 1) // P
    last = npc - (nblk - 1) * P
    assert SEG % P == 0
    return dict(
        N=N, G=G, H=H, NV=NV, SEG=SEG, NPC=npc, NBLK=nblk, LAST=last,
        NSEG=(npc + SEG - 1) // SEG,
        NVC=(NV + P - 1) // P,
        AGG_ROWS=nblk * P,
    )


# ---------------------------------------------------------------------------
# host-side preprocessing
# ---------------------------------------------------------------------------

def _pack_tiles(run_len):
    """Pack runs (all of one src-segment, dst-sorted) into tiles of <=128
    edges and <=J runs, runs kept whole."""
    K = len(run_len)
    tile_of_run = np.empty(K, np.int64)
    slot_of_run = np.empty(K, np.int64)
    t = 0
    edges = 0
    runs = 0
    for k in range(K):
        r = int(run_len[k])
        if r > P:
            raise ValueError(f"in-degree run {r} exceeds {P}")
        if edges + r > P or runs >= J:
            t += 1
            edges = 0
            runs = 0
        tile_of_run[k] = t
        slot_of_run[k] = runs
        edges += r
        runs += 1
    return tile_of_run, slot_of_run, t + 1


def _prep_core(cfg, cc, src, dst, in_feat, invdeg):
    npc, nblk = cfg["NPC"], cfg["NBLK"]
    SEG, NSEG = cfg["SEG"], cfg["NSEG"]
    lo = cc * npc
    sel = (dst >= lo) & (dst < lo + npc)
    e_src = src[sel]
    e_dstl = (dst[sel] - lo).astype(np.int64)
    e_seg = e_dstl // SEG
    order = np.lexsort((e_dstl, e_seg))
    e_src = e_src[order].astype(np.int64)
    e_dstl = e_dstl[order]
    e_seg = e_seg[order]
    Ec = len(e_src)

    # runs of equal (seg, dst)
    key = e_seg * npc + e_dstl
    change = np.empty(Ec, bool)
    if Ec:
        change[0] = True
        change[1:] = key[1:] != key[:-1]
    run_starts = np.nonzero(change)[0]
    run_len = np.diff(np.concatenate([run_starts, [Ec]]))
    run_dst = e_dstl[run_starts]
    run_seg = e_seg[run_starts]

    # pack per segment
    seg_tiles = []          # per segment: (tile_of_run idx arrays etc.)
    tiles_per_seg = np.zeros(NSEG, np.int64)
    seg_run_sel = [np.nonzero(run_seg == s)[0] for s in range(NSEG)]
    packs = []
    for s in range(NSEG):
        rl = run_len[seg_run_sel[s]]
        if len(rl):
            tr, sr, T = _pack_tiles(rl)
        else:
            tr = np.empty(0, np.int64)
            sr = np.empty(0, np.int64)
            T = 0
        packs.append((tr, sr))
        tiles_per_seg[s] = T
    return dict(
        Ec=Ec, e_src=e_src, e_dstl=e_dstl, run_starts=run_starts,
        run_len=run_len, run_dst=run_dst, run_seg=run_seg,
        seg_run_sel=seg_run_sel, packs=packs, tiles_per_seg=tiles_per_seg,
        in_feat=in_feat, invdeg=invdeg, lo=lo,
    )


def _finish_core(cfg, core, calls_per_seg):
    npc, nblk = cfg["NPC"], cfg["NBLK"]
    SEG, NSEG, NVC = cfg["SEG"], cfg["NSEG"], cfg["NVC"]
    ncalls = int(calls_per_seg.sum())
    T_total = ncalls * TPC
    SCR = cfg["SEG"]  # scratch row, local to each range's agg tensor

    src_idx = np.zeros((T_total, P), np.int64)   # within-segment row idx
    ind = np.zeros((T_total, P, J), np.float32)
    slotrow = np.full((T_total, J), SCR, np.int64)

    call_base_of_seg = np.concatenate([[0], np.cumsum(calls_per_seg)])[:-1]
    invdeg = core["invdeg"]
    for s in range(NSEG):
        rsel = core["seg_run_sel"][s]
        if not len(rsel):
            continue
        tr, sr = core["packs"][s]
        tile_base = call_base_of_seg[s] * TPC
        run_starts = core["run_starts"][rsel]
        run_len = core["run_len"][rsel]
        run_dst = core["run_dst"][rsel]
        # per-edge position info (vectorized)
        n_e = int(run_len.sum())
        roe = np.repeat(np.arange(len(rsel)), run_len)     # run-of-edge (local)
        t_e = tr[roe] + tile_base
        # first edge index (global, within e_src) of each tile
        e_idx = np.repeat(run_starts, run_len) + (
            np.arange(n_e) - np.repeat(np.cumsum(run_len) - run_len, run_len))
        tfe = np.full(tr.max() + 1 + tile_base, 1 << 60, np.int64)
        np.minimum.at(tfe, t_e, e_idx)
        pos_e = e_idx - tfe[t_e]
        j_e = sr[roe]
        src_idx[t_e, pos_e] = core["e_src"][e_idx]
        dst_e = core["e_dstl"][e_idx]
        ind[t_e, pos_e, j_e] = invdeg[core["lo"] + dst_e]
        slotrow[tr + tile_base, sr] = run_dst - s * SEG

    # ---- device layouts (walrus indirect DMA: one int32 offset column per
    # instruction; gather per tile, scatter per 128-slot PSUM chunk) ----
    src32 = np.ascontiguousarray(src_idx.T.astype(np.int32))  # [128, T_total]
    # scatter chunk (call c, psum q): row p holds slot (tile c*TPC+q*TPP+p//J,
    # run p%J) for p < TPP*J, scratch otherwise
    scat_tok = np.full((ncalls, NSLOT), SCR, np.int64)
    sr_view = slotrow.reshape(ncalls, TPC, J)
    for q in range(PPC):
        blk = sr_view[:, q * TPP:(q + 1) * TPP, :].reshape(ncalls, TPP * J)
        scat_tok[:, q * P:q * P + TPP * J] = blk
    scat32 = np.ascontiguousarray(
        scat_tok.reshape(ncalls * PPC, P).T.astype(np.int32))  # [128, ncalls*PPC]

    ind_dev = np.ascontiguousarray(
        ind.transpose(1, 0, 2).reshape(P, T_total * J))

    # inverse slot map: own node row -> slot-space index (or the zero row)
    seg_of_call = np.repeat(np.arange(NSEG), calls_per_seg)
    ZROW = ncalls * NSLOT
    d2s = np.full(nblk * P, ZROW, np.int64)
    tt, jj = np.nonzero(slotrow != SCR)
    loc = slotrow[tt, jj] + seg_of_call[tt // TPC] * SEG
    q_ = (tt % TPC) // TPP
    p_ = ((tt % TPC) % TPP) * J + jj
    d2s[loc] = (tt // TPC) * NSLOT + q_ * P + p_
    d2s_dev = np.ascontiguousarray(d2s.reshape(nblk, P).T.astype(np.int32))

    # layer-1 count matrix (1/deg folded)
    NV = cfg["NV"]
    v_e = core["in_feat"][core["e_src"]]
    cntm = np.zeros(npc * NVC * P, np.float32)
    np.add.at(cntm, core["e_dstl"] * (NVC * P) + v_e, 1.0)
    cntm = cntm.reshape(npc, NVC * P) * invdeg[core["lo"]:core["lo"] + npc,
                                               None].astype(np.float32)
    cnt_pad = np.zeros((nblk * P, NVC * P), np.float32)
    cnt_pad[:npc] = cntm
    cb = cnt_pad.reshape(nblk, P, NVC, P)
    cnt_dev = np.ascontiguousarray(
        cb.transpose(0, 3, 2, 1).reshape(nblk, P, NVC * P))

    # layer-1 self gather indices (emb rows per own node) [128, nblk]
    feat = np.zeros(nblk * P, np.int64)
    feat[:npc] = core["in_feat"][core["lo"]:core["lo"] + npc]
    feat32 = np.ascontiguousarray(feat.reshape(nblk, P).T.astype(np.int32))

    return dict(src32=src32, scat32=scat32, ind=ind_dev, cnt=cnt_dev,
                feat32=feat32, d2s=d2s_dev, ncalls0=0)


def _prep(cfg, in_feat, src, dst, graph_ids, emb, W_self, W_neigh, b,
          gamma, beta, prelu_w):
    N, G, H = cfg["N"], cfg["G"], cfg["H"]
    npc, nblk = cfg["NPC"], cfg["NBLK"]
    NV, NVC, NSEG = cfg["NV"], cfg["NVC"], cfg["NSEG"]
    in_feat = np.asarray(in_feat).astype(np.int64)
    src = np.asarray(src).astype(np.int64)
    dst = np.asarray(dst).astype(np.int64)
    graph_ids = np.asarray(graph_ids).astype(np.int64)

    deg = np.bincount(dst, minlength=N)
    invdeg = (1.0 / np.clip(deg, 1, None)).astype(np.float64)

    cores = [_prep_core(cfg, cc, src, dst, in_feat, invdeg)
             for cc in range(N_CORES)]
    calls_per_seg = np.zeros(NSEG, np.int64)
    for s in range(NSEG):
        mx = max(int(c["tiles_per_seg"][s]) for c in cores)
        calls_per_seg[s] = (mx + TPC - 1) // TPC
    ncalls = int(calls_per_seg.sum())

    fins = [_finish_core(cfg, c, calls_per_seg) for c in cores]

    cnt_g = np.clip(np.bincount(graph_ids, minlength=G), 1, None)
    emb_pad = np.zeros((NVC * P, H), np.float32)
    emb_pad[:NV] = np.asarray(emb, np.float32)

    # per-call dst-range index (same on every core)
    seg_bounds = []
    for s in range(NSEG):
        seg_bounds += [s] * int(calls_per_seg[s])

    in_maps = []
    for cc, fin in enumerate(fins):
        lo = cc * npc
        gown = np.zeros(nblk * P, np.int64)
        gown[:npc] = graph_ids[lo:lo + npc]
        gind = np.zeros((nblk * P, G), np.float32)
        gind[np.arange(npc), gown[:npc]] = 1.0 / cnt_g[gown[:npc]]
        gind = np.ascontiguousarray(gind.reshape(nblk, P, G))

        in_maps.append(dict(
            src32=fin["src32"], scat32=fin["scat32"], ind=fin["ind"],
            cnt=fin["cnt"], feat32=fin["feat32"], d2s=fin["d2s"],
            gind=gind, emb=emb_pad,
            zeros=np.zeros((cfg["SEG"] + P, H), np.float32),
            W_self=np.ascontiguousarray(np.asarray(W_self, np.float32)),
            W_neigh=np.ascontiguousarray(np.asarray(W_neigh, np.float32)),
            b_cols=np.ascontiguousarray(np.asarray(b, np.float32).T),
            gam_cols=np.ascontiguousarray(np.asarray(gamma, np.float32).T),
            bet_cols=np.ascontiguousarray(np.asarray(beta, np.float32).T),
            alp_cols=np.ascontiguousarray(np.asarray(prelu_w, np.float32).T),
        ))
    return in_maps, ncalls, seg_bounds, fins[0]["ncalls0"]


# ---------------------------------------------------------------------------
# device program
# ---------------------------------------------------------------------------

def build_program(cfg, ncalls, seg_bounds, ncalls0, ablate=()):
    N, G, H = cfg["N"], cfg["G"], cfg["H"]
    npc, nblk, last = cfg["NPC"], cfg["NBLK"], cfg["LAST"]
    NVC = cfg["NVC"]
    agg_rows = cfg["AGG_ROWS"]
    T_total = ncalls * TPC

    nc = bacc.Bacc("TRN2", target_bir_lowering=False, debug=False,
                   num_devices=N_CORES)

    src32_d = nc.declare_dram_parameter("src32", [P, T_total], I32,
                                        isOutput=False)
    scat32_d = nc.declare_dram_parameter("scat32", [P, ncalls * PPC], I32,
                                         isOutput=False)
    ind_d = nc.declare_dram_parameter("ind", [P, T_total * J], FP, isOutput=False)
    cnt_d = nc.declare_dram_parameter("cnt", [nblk, P, NVC * P], FP, isOutput=False)
    feat32_d = nc.declare_dram_parameter("feat32", [P, nblk], I32,
                                         isOutput=False)
    d2s_d = nc.declare_dram_parameter("d2s", [P, nblk], I32, isOutput=False)
    gind_d = nc.declare_dram_parameter("gind", [nblk, P, G], FP, isOutput=False)
    emb_d = nc.declare_dram_parameter("emb", [NVC * P, H], FP, isOutput=False)
    zeros_d = nc.declare_dram_parameter("zeros", [cfg["SEG"] + P, H], FP,
                                        isOutput=False)
    ws_d = nc.declare_dram_parameter("W_self", [L, H, H], FP, isOutput=False)
    wn_d = nc.declare_dram_parameter("W_neigh", [L, H, H], FP, isOutput=False)
    bcol_d = nc.declare_dram_parameter("b_cols", [H, L], FP, isOutput=False)
    gcol_d = nc.declare_dram_parameter("gam_cols", [H, L], FP, isOutput=False)
    becol_d = nc.declare_dram_parameter("bet_cols", [H, L], FP, isOutput=False)
    acol_d = nc.declare_dram_parameter("alp_cols", [H, L], FP, isOutput=False)
    out_d = nc.declare_dram_parameter("out", [G, L * H], FP, isOutput=True)

    NSEG = cfg["NSEG"]
    SEG = cfg["SEG"]
    h_shard = nc.dram_tensor("h_shard", [npc, H], FP)
    h_full = nc.dram_tensor("h_full", [N, H], FP, addr_space="Shared")
    # slot-space aggregation buffer + one trailing zero block for deg-0 rows
    slotbuf = nc.dram_tensor("slotbuf", [ncalls * NSLOT + P, H], FP)
    stats_loc = nc.dram_tensor("stats_loc", [2, H], FP)
    stats_red = nc.dram_tensor("stats_red", [2, H], FP, addr_space="Shared")
    pool_loc = nc.dram_tensor("pool_loc", [L, G, H], FP)
    pool_red = nc.dram_tensor("pool_red", [L, G, H], FP, addr_space="Shared")

    groups = [list(range(N_CORES))]

    with tile.TileContext(nc) as tc:
        with (
            tc.tile_pool(name="res", bufs=1) as res,
            tc.tile_pool(name="wrk", bufs=3) as wrk,
            tc.tile_pool(name="gat", bufs=3) as gat,
            tc.tile_pool(name="ps_slot", bufs=3, space="PSUM") as ps_slot,
            tc.tile_pool(name="ps_tp", bufs=2, space="PSUM") as ps_tp,
            tc.tile_pool(name="ps_rst", bufs=2, space="PSUM") as ps_rst,
            tc.tile_pool(name="ps_pool", bufs=1, space="PSUM") as ps_pool,
        ):
            ident = res.tile([P, P], FP, tag="ident")
            make_identity(nc, ident[:])

            src32_sb = res.tile([P, T_total], I32, tag="src32")
            nc.sync.dma_start(src32_sb[:], src32_d[:])
            scat32_sb = res.tile([P, ncalls * PPC], I32, tag="scat32")
            nc.sync.dma_start(scat32_sb[:], scat32_d[:])
            feat32_sb = res.tile([P, nblk], I32, tag="feat32")
            nc.sync.dma_start(feat32_sb[:], feat32_d[:])
            d2s_sb = res.tile([P, nblk], I32, tag="d2s")
            nc.sync.dma_start(d2s_sb[:], d2s_d[:])
            emb_sb = res.tile([P, NVC * H], FP, tag="emb")
            for c in range(NVC):
                nc.sync.dma_start(emb_sb[:, c * H:(c + 1) * H],
                                  emb_d[c * P:(c + 1) * P, :])
            ws_sb = res.tile([P, L * H], FP, tag="ws")
            wn_sb = res.tile([P, L * H], FP, tag="wn")
            for l in range(L):
                nc.sync.dma_start(ws_sb[:, l * H:(l + 1) * H], ws_d[l])
                nc.sync.dma_start(wn_sb[:, l * H:(l + 1) * H], wn_d[l])
            bcol_sb = res.tile([P, L], FP, tag="bcol")
            nc.sync.dma_start(bcol_sb[:], bcol_d[:])
            gcol_sb = res.tile([P, L], FP, tag="gcol")
            nc.sync.dma_start(gcol_sb[:], gcol_d[:])
            becol_sb = res.tile([P, L], FP, tag="becol")
            nc.sync.dma_start(becol_sb[:], becol_d[:])
            acol_sb = res.tile([P, L], FP, tag="acol")
            nc.sync.dma_start(acol_sb[:], acol_d[:])

            h_stage = res.tile([P, nblk * P], FP, tag="hstage")
            stats_sum = res.tile([P, nblk], FP, tag="ssum")
            stats_sq = res.tile([P, nblk], FP, tag="ssq")
            scratch = res.tile([P, P], FP, tag="scratch")
            eps_col = res.tile([P, 1], FP, tag="eps")
            nc.vector.memset(eps_col[:], float(EPS))

            # S staging: partitions [TPP*J:128) of each chunk feed zeros
            S_bufs = []
            for i in range(4):
                Sb = res.tile([P, PPC * H], FP, tag=f"S{i}")
                nc.vector.memset(Sb[TPP * J:, :], 0.0)
                S_bufs.append(Sb)

            # zero the trailing slotbuf block once (deg-0 rows point here)
            nc.sync.dma_start(slotbuf[ncalls * NSLOT:, :], zeros_d[:P, :])

            for l in range(L):
                # ---------------- aggregation ----------------
                if l > 0:
                    for c in range(ncalls):
                        gt = gat.tile([P, TPC * H], FP, tag="g")
                        if "gather" not in ablate:
                            for ti in range(TPC):
                                t_glob = c * TPC + ti
                                nc.gpsimd.indirect_dma_start(
                                    out=gt[:, ti * H:(ti + 1) * H],
                                    out_offset=None, in_=h_full[:],
                                    in_offset=bass.IndirectOffsetOnAxis(
                                        ap=src32_sb[:, t_glob:t_glob + 1],
                                        axis=0))
                        it = wrk.tile([P, TPC * J], FP, tag="indblk")
                        nc.sync.dma_start(
                            it[:], ind_d[:, c * TPC * J:(c + 1) * TPC * J])
                        S = S_bufs[c % 4]
                        for q in range(PPC):
                            ps = ps_slot.tile([P, H], FP, tag="slot")
                            if "aggmm" not in ablate:
                                for ti in range(TPP):
                                    t_loc = q * TPP + ti
                                    nc.tensor.matmul(
                                        ps[ti * J:(ti + 1) * J, :],
                                        lhsT=it[:, t_loc * J:(t_loc + 1) * J],
                                        rhs=gt[:, t_loc * H:(t_loc + 1) * H],
                                        start=True, stop=True)
                                nc.vector.tensor_copy(
                                    S[:TPP * J, q * H:(q + 1) * H],
                                    ps[:TPP * J, :])
                        if "scatter" not in ablate:
                            # one direct HWDGE DMA flushes all 1024 slots
                            nc.sync.dma_start(
                                slotbuf[c * NSLOT:(c + 1) * NSLOT, :]
                                .rearrange("(q p) f -> p q f", p=P),
                                S[:].rearrange("p (q f) -> p q f", f=H))

                # ---------------- main (pass A) ----------------
                for bI in range(nblk):
                    nn = last if bI == nblk - 1 else P
                    ab = wrk.tile([P, H], FP, tag="mablk")
                    if l == 0:
                        cnt_sb = wrk.tile([P, NVC * H], FP, tag="cntblk")
                        nc.sync.dma_start(cnt_sb[:], cnt_d[bI])
                        ps_a = ps_rst.tile([P, H], FP, tag="rst")
                        for cv in range(NVC):
                            nc.tensor.matmul(
                                ps_a[:],
                                lhsT=cnt_sb[:, cv * H:(cv + 1) * H],
                                rhs=emb_sb[:, cv * H:(cv + 1) * H],
                                start=(cv == 0), stop=(cv == NVC - 1))
                        nc.vector.tensor_copy(ab[:], ps_a[:])
                    else:
                        nc.gpsimd.indirect_dma_start(
                            out=ab[:], out_offset=None, in_=slotbuf[:],
                            in_offset=bass.IndirectOffsetOnAxis(
                                ap=d2s_sb[:, bI:bI + 1], axis=0))
                    ps_t = ps_tp.tile([P, P], FP, tag="tp")
                    nc.tensor.transpose(out=ps_t[:], in_=ab[:], identity=ident[:])
                    aT = wrk.tile([P, P], FP, tag="aT")
                    nc.scalar.copy(aT[:], ps_t[:])

                    if l == 0:
                        g0 = wrk.tile([P, H], FP, tag="g0")
                        nc.gpsimd.indirect_dma_start(
                            out=g0[:], out_offset=None, in_=emb_d[:],
                            in_offset=bass.IndirectOffsetOnAxis(
                                ap=feat32_sb[:, bI:bI + 1], axis=0))
                        ps_t0 = ps_tp.tile([P, P], FP, tag="tp")
                        nc.tensor.transpose(out=ps_t0[:], in_=g0[:],
                                            identity=ident[:])
                        hT = wrk.tile([P, P], FP, tag="hT")
                        nc.scalar.copy(hT[:], ps_t0[:])
                        rhs_self = hT[:]
                    else:
                        rhs_self = h_stage[:, bI * P:(bI + 1) * P]

                    ps_r = ps_rst.tile([P, H], FP, tag="rst")
                    nc.tensor.matmul(ps_r[:], lhsT=ws_sb[:, l * H:(l + 1) * H],
                                     rhs=rhs_self, start=True, stop=False)
                    nc.tensor.matmul(ps_r[:], lhsT=wn_sb[:, l * H:(l + 1) * H],
                                     rhs=aT[:], start=False, stop=True)

                    bc = bcol_sb[:, l:l + 1]
                    t1 = wrk.tile([P, P], FP, tag="t1")
                    nc.scalar.activation(t1[:], ps_r[:],
                                         mybir.ActivationFunctionType.Relu,
                                         bias=bc)
                    neg = wrk.tile([P, P], FP, tag="neg")
                    nc.vector.tensor_scalar(
                        neg[:], ps_r[:], bc, 0.0,
                        op0=mybir.AluOpType.add, op1=mybir.AluOpType.min)
                    if nn == P:
                        nc.vector.scalar_tensor_tensor(
                            h_stage[:, bI * P:(bI + 1) * P],
                            neg[:], acol_sb[:, l:l + 1], t1[:],
                            op0=mybir.AluOpType.mult, op1=mybir.AluOpType.add,
                            accum_out=stats_sum[:, bI:bI + 1])
                        nc.scalar.activation(scratch[:],
                                             h_stage[:, bI * P:(bI + 1) * P],
                                             mybir.ActivationFunctionType.Square,
                                             accum_out=stats_sq[:, bI:bI + 1])
                    else:
                        nc.vector.scalar_tensor_tensor(
                            h_stage[:, bI * P:bI * P + nn],
                            neg[:, :nn], acol_sb[:, l:l + 1], t1[:, :nn],
                            op0=mybir.AluOpType.mult, op1=mybir.AluOpType.add,
                            accum_out=stats_sum[:, bI:bI + 1])
                        nc.vector.scalar_tensor_tensor(
                            h_stage[:, bI * P + nn:(bI + 1) * P],
                            neg[:, nn:], acol_sb[:, l:l + 1], t1[:, nn:],
                            op0=mybir.AluOpType.mult, op1=mybir.AluOpType.add)
                        nc.scalar.activation(
                            scratch[:, :nn], h_stage[:, bI * P:bI * P + nn],
                            mybir.ActivationFunctionType.Square,
                            accum_out=stats_sq[:, bI:bI + 1])

                # ---------------- BN stats + allreduce ----------------
                sx = wrk.tile([P, 1], FP, tag="sx")
                nc.vector.tensor_reduce(sx[:], stats_sum[:],
                                        axis=mybir.AxisListType.X,
                                        op=mybir.AluOpType.add)
                sq = wrk.tile([P, 1], FP, tag="sq")
                nc.vector.tensor_reduce(sq[:], stats_sq[:],
                                        axis=mybir.AxisListType.X,
                                        op=mybir.AluOpType.add)
                nc.sync.dma_start(stats_loc[0:1, :], sx[:, 0:1])
                nc.sync.dma_start(stats_loc[1:2, :], sq[:, 0:1])
                nc.gpsimd.collective_compute(
                    "AllReduce", mybir.AluOpType.add, replica_groups=groups,
                    ins=[stats_loc[:]], outs=[stats_red[:]])
                sxr = wrk.tile([P, 1], FP, tag="sxr")
                nc.sync.dma_start(sxr[:, 0:1], stats_red[0:1, :])
                sqr = wrk.tile([P, 1], FP, tag="sqr")
                nc.sync.dma_start(sqr[:, 0:1], stats_red[1:2, :])

                mu = wrk.tile([P, 1], FP, tag="mu")
                nc.scalar.mul(mu[:], sxr[:], 1.0 / N)
                ex2 = wrk.tile([P, 1], FP, tag="ex2")
                nc.scalar.mul(ex2[:], sqr[:], 1.0 / N)
                mu2 = wrk.tile([P, 1], FP, tag="mu2")
                nc.scalar.square(mu2[:], mu[:])
                var = wrk.tile([P, 1], FP, tag="var")
                nc.vector.tensor_sub(var[:], ex2[:], mu2[:])
                sd = wrk.tile([P, 1], FP, tag="sd")
                nc.scalar.activation(sd[:], var[:],
                                     mybir.ActivationFunctionType.Sqrt,
                                     bias=eps_col[:])
                rstd = wrk.tile([P, 1], FP, tag="rstd")
                nc.vector.reciprocal(rstd[:], sd[:])
                scale = wrk.tile([P, 1], FP, tag="scale")
                nc.vector.tensor_mul(scale[:], rstd[:], gcol_sb[:, l:l + 1])
                msc = wrk.tile([P, 1], FP, tag="msc")
                nc.vector.tensor_mul(msc[:], mu[:], scale[:])
                shift = wrk.tile([P, 1], FP, tag="shift")
                nc.vector.tensor_sub(shift[:], becol_sb[:, l:l + 1], msc[:])

                # ---------------- pass B ----------------
                ps_p = ps_pool.tile([P, H], FP, tag="pool")
                for bI in range(nblk):
                    nn = last if bI == nblk - 1 else P
                    sl = h_stage[:, bI * P:(bI + 1) * P]
                    nc.vector.scalar_tensor_tensor(
                        sl, sl, scale[:], shift[:].to_broadcast([P, P]),
                        op0=mybir.AluOpType.mult, op1=mybir.AluOpType.add)
                    ps_t = ps_tp.tile([P, P], FP, tag="tp")
                    nc.tensor.transpose(out=ps_t[:], in_=sl, identity=ident[:])
                    hnm = wrk.tile([P, P], FP, tag="hnm")
                    nc.scalar.copy(hnm[:], ps_t[:])
                    if l < L - 1:
                        nc.sync.dma_start(
                            h_shard[bI * P:bI * P + nn, :], hnm[:nn, :])
                    gb = wrk.tile([P, G], FP, tag="gblk")
                    nc.sync.dma_start(gb[:], gind_d[bI])
                    nc.tensor.matmul(ps_p[:G, :], lhsT=gb[:], rhs=hnm[:],
                                     start=(bI == 0), stop=(bI == nblk - 1))
                pl = wrk.tile([P, H], FP, tag="pl")
                nc.vector.tensor_copy(pl[:G, :], ps_p[:G, :])
                nc.sync.dma_start(pool_loc[l], pl[:G, :])

                if l < L - 1:
                    nc.gpsimd.collective_compute(
                        "AllGather", mybir.AluOpType.bypass,
                        replica_groups=groups,
                        ins=[h_shard[:]], outs=[h_full[:]])

            nc.gpsimd.collective_compute(
                "AllReduce", mybir.AluOpType.add, replica_groups=groups,
                ins=[pool_loc[:]], outs=[pool_red[:]])
            for l in range(L):
                ob = wrk.tile([P, H], FP, tag="ob")
                nc.sync.dma_start(ob[:G, :], pool_red[l])
                nc.sync.dma_start(out_d[:, l * H:(l + 1) * H], ob[:G, :])

    nc.compile()
    return nc


# ---------------------------------------------------------------------------
# entry point
# ---------------------------------------------------------------------------

_CACHE = {}


def _run(cfg, inputs, trace=False):
    from concourse.bass_utils import run_bass_kernel_spmd
    in_maps, ncalls, seg_bounds, ncalls0 = _prep(cfg, **inputs)
    key = (cfg["N"], cfg["G"], cfg["H"], ncalls, tuple(seg_bounds))
    if key not in _CACHE:
        _CACHE[key] = build_program(cfg, ncalls, seg_bounds, ncalls0)
    nc = _CACHE[key]
    last_exc = None
    for attempt in range(3):
        try:
            return run_bass_kernel_spmd(nc, in_maps, list(range(N_CORES)),
                                        trace=trace)
        except Exception as e:  # rare transient device-unrecoverable errors
            last_exc = e
            try:
                import jax
                import jax.extend.backend
                jax.clear_caches()
                jax.extend.backend.clear_backends()
            except Exception:
                pass
    raise last_exc


def kernel(in_feat, src, dst, graph_ids, emb, W_self, W_neigh, b,
           gamma, beta, prelu_w):
    cfg = _mkcfg(**CFG_FULL)
    res = _run(cfg, dict(
        in_feat=in_feat, src=src, dst=dst, graph_ids=graph_ids, emb=emb,
        W_self=W_self, W_neigh=W_neigh, b=b, gamma=gamma, beta=beta,
        prelu_w=prelu_w))
    return np.asarray(res.results[0]["out"], np.float32)

